# revision 18
# baseline (speedup 1.0000x reference)
"""CenterLoss kernel for Trainium2 (8 NeuronCores, class-sharded data-parallel).

loss = sum((x - centers[labels])**2) / 2 / B

Strategy: expand the loss so no per-sample center gather is needed:
    sum_i ||x_i - c_{l_i}||^2 = sum_i ||x_i||^2
                              - 2 * sum_k <S_k, c_k>
                              + sum_k n_k ||c_k||^2
with S_k = sum of x_i whose label is k and n_k = count of label k.

Host: sort samples by label, cut into 8 shards at class boundaries so each
shard covers <= 128 consecutive classes (span fits one PSUM bank). Ship per
core: x shard in fp8 (sorted order, zero-padded to NSP), one-hot lhsT tiles
A (fp8), the 128-row local centers slice (bf16) and sqrt(counts) (f32).

Device per core:
  - sync ring DMAs x in chunks (fp8, [128, T, 512] p-major tiles).
  - PE: DoubleRow fp8 matmuls psum[m, d] += sum_p A[p, j, m] * x[p, j, d]
    accumulating S over all tiles into one PSUM bank (local class m on
    partitions, d along free).
  - ACT: Square w/ accum_out on a ~55% slice of each chunk (sum x^2).
  - DVE: tensor_tensor x*x (fp8 out) on the rest; PE reduces those squares
    with DoubleRow ones-matmuls into a second PSUM bank (tensor_tensor_reduce
    does not lower/run on this toolchain, so reductions go through PE).
  - cross term: tensor_tensor PSUM(S) x cen -> CP, then a ones-matmul.
  - count term: ACT Square(cen * sqrt(n)) w/ accum_out.
Host sums [128, nch+1] + two [1, 512] partials of all cores in f64.
"""

import sys

sys.path.insert(0, "/opt/trn_rl_repo")

from contextlib import ExitStack

import numpy as np

import concourse.bass as bass  # noqa: F401  (AP types)
from concourse import bacc, mybir
from concourse.bass_utils import run_bass_kernel_spmd

P = 128
D = 512
NCLASS = 1000
NCORES = 8
BATCH = 65536

FP8 = mybir.dt.float8e4
BF16 = mybir.dt.bfloat16
F32 = mybir.dt.float32

# fraction of each chunk's tiles squared on ACT (rest on DVE):
# ACT 153.6 G elem/s vs DVE 122.9 G elem/s at 1x (fp8) -> 0.555
ACT_FRAC = 0.58


def plan_chunks(nt: int) -> list[int]:
    """Split nt (even) tiles into even-sized chunks: small head for pipeline
    ramp, 16-tile body, 2-tile tail for a short drain."""
    assert nt % 2 == 0 and nt >= 4
    chunks = [min(8, nt - 2)]
    rem = nt - chunks[0]
    while rem > 0:
        t = min(16, rem)
        if rem - t == 0 and t > 4:
            chunks.append(t - 2)
            chunks.append(2)
        else:
            chunks.append(t)
        rem -= t
    assert sum(chunks) == nt and all(c % 2 == 0 for c in chunks)
    return chunks


def build(nsp: int, num_devices: int = NCORES, chunks=None, act_frac=None, nx=None):
    """Per-core Bass program; nsp = padded samples per core (mult of 256)."""
    nt = nsp // P
    chunks = plan_chunks(nt) if chunks is None else list(chunks)
    assert sum(chunks) == nt and all(c % 2 == 0 for c in chunks)
    nch = len(chunks)
    ncol = 2 * nch + 2  # [act sq | dve sq | cross | count-norm]

    nc = bacc.Bacc(
        "TRN2", target_bir_lowering=False, debug=False, num_devices=num_devices
    )
    x_d = nc.dram_tensor("x", [nsp, D], FP8, kind="ExternalInput")
    a_d = nc.dram_tensor("a", [P, nt * P], FP8, kind="ExternalInput")
    cen_d = nc.dram_tensor("cen", [P, D], BF16, kind="ExternalInput")
    sqn_d = nc.dram_tensor("sqn", [P, 1], F32, kind="ExternalInput")
    out_d = nc.dram_tensor("out", [P, nch + 1], F32, kind="ExternalOutput")
    out2_d = nc.dram_tensor("out2", [2, D], F32, kind="ExternalOutput")

    NX = 3 if nx is None else nx  # x chunk buffers in flight
    af = ACT_FRAC if act_frac is None else act_frac
    tmax = max(chunks)
    sa_list = [min(t - 1, max(1, round(t * af))) for t in chunks]
    bases = [P * sum(chunks[:c]) for c in range(nch)]
    gtile = [sum(chunks[:c]) for c in range(nch)]

    with ExitStack() as ctx:
        e = ctx.enter_context
        xt = [e(nc.sbuf_tensor(f"xt{i}", [P, tmax, D], FP8)) for i in range(NX)]
        a_sb = e(nc.sbuf_tensor("a_sb", [P, nt, P], FP8))
        cen = e(nc.sbuf_tensor("cen_sb", [P, D], BF16))
        sqn = e(nc.sbuf_tensor("sqn_sb", [P, 1], F32))
        scr = e(nc.sbuf_tensor("scr", [P, nt, D], BF16))
        ones8 = e(nc.sbuf_tensor("ones8", [P, 2, 1], BF16))
        wt = e(nc.sbuf_tensor("wt", [P, 64], BF16))
        onesb = e(nc.sbuf_tensor("onesb", [P, 1], BF16))
        cp = e(nc.sbuf_tensor("cp", [P, D], BF16))
        red2 = e(nc.sbuf_tensor("red2", [1, D], F32))
        red3 = e(nc.sbuf_tensor("red3", [1, D], F32))
        scr_c = e(nc.sbuf_tensor("scr_c", [P, D], BF16))
        acc = e(nc.sbuf_tensor("acc", [P, nch + 1], F32))
        psum = e(nc.psum_tensor("S", [P, D], F32))
        psum2 = e(nc.psum_tensor("S2", [1, D], F32))
        psum3 = e(nc.psum_tensor("S3", [1, D], F32))

        s_ca = e(nc.semaphore("s_ca"))
        s_ca2 = e(nc.semaphore("s_ca2"))
        s_cc = e(nc.semaphore("s_cc"))
        s_cn = e(nc.semaphore("s_cn"))
        s_xa = [e(nc.semaphore(f"s_xa{c}")) for c in range(nch)]
        s_xb = [e(nc.semaphore(f"s_xb{c}")) for c in range(nch)]
        s_pe = e(nc.semaphore("s_pe"))
        s_sq = e(nc.semaphore("s_sq"))
        s_tt = e(nc.semaphore("s_tt"))
        s_on = e(nc.semaphore("s_on"))
        s_cp = e(nc.semaphore("s_cp"))
        s_ps2 = e(nc.semaphore("s_ps2"))
        s_ps3 = e(nc.semaphore("s_ps3"))
        s_red = e(nc.semaphore("s_red"))
        s_out = e(nc.semaphore("s_out"))

        npairs = nt // 2

        blk = ctx.enter_context(nc.Block())

        def chunk_dma(eng, c, t, sa):
            x_r = x_d.ap()[bases[c] : bases[c] + P * t, :].rearrange(
                "(p t) d -> p t d", p=P
            )
            eng.dma_start(xt[c % NX][:, :sa], x_r[:, :sa]).then_inc(s_xa[c], 16)
            eng.dma_start(xt[c % NX][:, sa:t], x_r[:, sa:t]).then_inc(s_xb[c], 16)

        @blk.gpsimd
        def _(gpsimd):
            gpsimd.memset(ones8[:], 1.0).then_inc(s_on, 1)
            gpsimd.memset(onesb[:], 1.0).then_inc(s_on, 1)
            gpsimd.memset(wt[:], 1.0).then_inc(s_on, 1)
            for c, t in enumerate(chunks):
                if c % 2 != 1:
                    continue
                if c >= NX:
                    gpsimd.wait_ge(s_pe, c - NX + 1)
                    gpsimd.wait_ge(s_sq, c - NX + 1)
                    gpsimd.wait_ge(s_tt, c - NX + 1)
                chunk_dma(gpsimd, c, t, sa_list[c])

        @blk.scalar
        def _(scalar):
            # cfg DMAs on the ACT HWDGE ring (separate FIFO from x loads)
            asp = gtile[min(2, nch - 1)]
            scalar.dma_start(a_sb[:, :asp], a_d.ap()[:, : asp * P]).then_inc(s_ca, 16)
            scalar.dma_start(a_sb[:, asp:], a_d.ap()[:, asp * P :]).then_inc(s_ca2, 16)
            scalar.dma_start(cen[:], cen_d.ap()).then_inc(s_cc, 16)
            scalar.dma_start(sqn[:], sqn_d.ap()).then_inc(s_cn, 16)
            for c, t in enumerate(chunks):
                sa = sa_list[c]
                scalar.wait_ge(s_xa[c], 16)  # ACT tiles are DMA half 1
                scalar.activation(
                    scr[:, gtile[c] : gtile[c] + sa],
                    xt[c % NX][:, :sa],
                    mybir.ActivationFunctionType.Square,
                    accum_out=acc[:, c : c + 1],
                ).then_inc(s_sq, 1)
            # count-norm term: Square(cen * sqrt(n)) summed over d
            scalar.wait_ge(s_cc, 16)
            scalar.wait_ge(s_cn, 16)
            scalar.activation(
                scr_c[:],
                cen[:],
                mybir.ActivationFunctionType.Square,
                scale=sqn[:, 0:1],
                accum_out=acc[:, nch : nch + 1],
            ).then_inc(s_sq, 1)

        @blk.sync
        def _(sync):
            for c, t in enumerate(chunks):
                if c % 2 != 0:
                    continue
                if c >= NX:
                    # buffer c % NX free once chunk c-NX fully consumed
                    sync.wait_ge(s_pe, c - NX + 1)
                    sync.wait_ge(s_sq, c - NX + 1)
                    sync.wait_ge(s_tt, c - NX + 1)
                chunk_dma(sync, c, t, sa_list[c])
            sync.wait_ge(s_red, 1)
            sync.dma_start(out2_d.ap()[0:1, :], red2[:]).then_inc(s_out, 16)
            sync.dma_start(out2_d.ap()[1:2, :], red3[:]).then_inc(s_out, 16)
            sync.wait_ge(s_sq, nch + 1)
            sync.dma_start(out_d.ap(), acc[:]).then_inc(s_out, 16)
            sync.wait_ge(s_out, 48)

        # ones-reduction matmul sequences over scr (DVE squares), per chunk
        ones_jobs = []  # per chunk: list of (g_start, width) with width in {1,2}
        for c, t in enumerate(chunks):
            sa = sa_list[c]
            jobs = [(gtile[c] + u, 1) for u in range(sa, t)]
            ones_jobs.append(jobs)
        n_ones = sum(len(j) for j in ones_jobs)

        @blk.tensor
        def _(tensor):
            # warmup spin: hold the PE p-state up while DMAs fill
            tensor.wait_ge(s_on, 3)
            for _ in range(24):
                tensor.matmul(
                    psum3[:, 0:64],
                    lhsT=onesb[:],
                    rhs=wt[:],
                    start=True,
                    stop=True,
                    skip_group_check=True,
                )
            tensor.wait_ge(s_ca, 16)
            pair = 0
            kone = 0

            def emit_ones(tensor, c):
                nonlocal kone
                tensor.wait_ge(s_tt, c + 1)
                for g, w in ones_jobs[c]:
                    mmo = tensor.matmul(
                        psum2[:],
                        lhsT=ones8[:, 0, :],
                        rhs=scr[:, g],
                        start=(kone == 0),
                        stop=(kone == n_ones - 1),
                        skip_group_check=True,
                    )
                    if kone == n_ones - 1:
                        mmo.then_inc(s_ps2, 1)
                    kone += 1

            for c, t in enumerate(chunks):
                sa = sa_list[c]
                half1_pairs = sa // 2
                if c == 2:
                    tensor.wait_ge(s_ca2, 16)
                tensor.wait_ge(s_xa[c], 16)
                for j in range(t // 2):
                    if j == half1_pairs:
                        tensor.wait_ge(s_xb[c], 16)
                    mm = tensor.matmul(
                        psum[:],
                        lhsT=a_sb[:, gtile[c] + 2 * j : gtile[c] + 2 * j + 2, :],
                        rhs=xt[c % NX][:, 2 * j : 2 * j + 2, :],
                        start=(pair == 0),
                        stop=(pair == npairs - 1),
                        perf_mode=mybir.MatmulPerfMode.DoubleRow,
                        skip_group_check=True,
                    )
                    pair += 1
                mm.then_inc(s_pe, 1)
                if c >= 1:
                    emit_ones(tensor, c - 1)
            emit_ones(tensor, nch - 1)
            # cross term reduction
            tensor.wait_ge(s_cp, 1)
            tensor.matmul(
                psum3[:], lhsT=onesb[:], rhs=cp[:], start=True, stop=True
            ).then_inc(s_ps3, 1)

        @blk.vector
        def _(vector):
            for c, t in enumerate(chunks):
                sa = sa_list[c]
                vector.wait_ge(s_xb[c], 16)
                vector.tensor_tensor(
                    scr[:, gtile[c] + sa : gtile[c] + t],
                    xt[c % NX][:, sa:t],
                    xt[c % NX][:, sa:t],
                    mybir.AluOpType.mult,
                ).then_inc(s_tt, 1)
            # cross term: CP = S (PSUM) * cen, reduced by PE ones-matmul
            vector.wait_ge(s_pe, nch)
            vector.wait_ge(s_cc, 16)
            vector.tensor_tensor(
                cp[:], psum[:], cen[:], mybir.AluOpType.mult
            ).then_inc(s_cp, 1)
            vector.wait_ge(s_ps2, 1)
            vector.tensor_copy(red2[:], psum2[:])
            vector.wait_ge(s_ps3, 1)
            vector.tensor_copy(red3[:], psum3[:]).then_inc(s_red, 1)

    nc.compile()
    return nc, chunks


def _shard(labels: np.ndarray):
    """Class-contiguous cuts with span <= 128 per shard, near count octiles."""
    cnt = np.bincount(labels, minlength=NCLASS)
    cum = np.concatenate([[0], np.cumsum(cnt)])
    cuts = [0]
    for i in range(1, NCORES):
        tgt = BATCH * i // NCORES
        k = int(np.searchsorted(cum, tgt))
        if k > 0 and abs(int(cum[k - 1]) - tgt) < abs(int(cum[k]) - tgt):
            k -= 1
        k = max(k, cuts[-1] + 1)
        k = max(k, NCLASS - (NCORES - i) * P)  # leave room for later shards
        k = min(k, cuts[-1] + P)
        cuts.append(k)
    cuts.append(NCLASS)
    spans = [cuts[i + 1] - cuts[i] for i in range(NCORES)]
    assert all(0 < s <= P for s in spans), f"class spans {spans} exceed {P}"
    return cuts, cum


_NC = {}


def run(x, labels, centers, **spmd_kwargs):
    import ml_dtypes

    fp8 = ml_dtypes.float8_e4m3fn
    bf16 = ml_dtypes.bfloat16

    x = np.ascontiguousarray(np.asarray(x, dtype=np.float32))
    labels = np.asarray(labels).astype(np.int64)
    centers = np.asarray(centers, dtype=np.float32)

    order = np.argsort(labels, kind="stable")
    ls = labels[order]
    cuts, cum = _shard(ls)
    lo = [int(cum[cuts[i]]) for i in range(NCORES)]
    hi = [int(cum[cuts[i + 1]]) for i in range(NCORES)]
    max_n = max(h - l for l, h in zip(lo, hi))
    nsp = ((max_n + 255) // 256) * 256
    nt = nsp // P

    key = nsp
    if key not in _NC:
        _NC[key] = build(nsp)
    nc, chunks = _NC[key]

    x8 = x[order].astype(fp8)
    c16 = centers.astype(bf16)

    in_maps = []
    for i in range(NCORES):
        n = hi[i] - lo[i]
        k0, k1 = cuts[i], cuts[i + 1]

        xs = np.zeros((nsp, D), dtype=fp8)
        xs[:n] = x8[lo[i] : hi[i]]

        # local class per sorted-sample position, pad -> 255 (never matches)
        lloc = np.full(nsp, 255, dtype=np.int64)
        lloc[:n] = ls[lo[i] : hi[i]] - k0

        # one-hot lhsT tiles in the p-major chunk layout:
        # tile g (in chunk c of t tiles), A[p, g, m] = 1 iff
        # lloc[base_c + p*t + (g - g0)] == m
        a_u8 = np.zeros((P, nt, P), dtype=np.uint8)
        base = 0
        g0 = 0
        for t in chunks:
            lb = lloc[base : base + P * t].reshape(P, t)  # [p, tt]
            a_u8[:, g0 : g0 + t, :] = (
                lb[:, :, None] == np.arange(P)[None, None, :]
            ) * np.uint8(0x38)  # fp8e4m3 bit pattern of 1.0
            base += P * t
            g0 += t
        a8 = a_u8.view(fp8).reshape(P, nt * P)

        cenp = np.zeros((P, D), dtype=bf16)
        cenp[: k1 - k0] = c16[k0:k1]

        nk = np.bincount(lloc[:n], minlength=P).astype(np.float64)
        sqn = np.sqrt(nk[:P]).astype(np.float32).reshape(P, 1)

        in_maps.append({"x": xs, "a": a8, "cen": cenp, "sqn": sqn})

    res = run_bass_kernel_spmd(nc, in_maps, list(range(NCORES)), **spmd_kwargs)

    nch = len(chunks)
    total = 0.0
    for i in range(NCORES):
        o = res.results[i]["out"].astype(np.float64)
        o2 = res.results[i]["out2"].astype(np.float64)
        ss = o[:, :nch].sum() + o2[0].sum()
        cr = o2[1].sum()
        nm = o[:, nch].sum()
        total += ss - 2.0 * cr + nm
    loss = total / 2.0 / BATCH
    return np.array(loss, dtype=np.float32), res


def kernel(x: np.ndarray, labels: np.ndarray, centers: np.ndarray) -> np.ndarray:
    loss, _ = run(x, labels, centers)
    return loss
